# revision 15
# baseline (speedup 1.0000x reference)
"""MultiHeadAttention Trainium2 kernel (8 NeuronCores).

Sharding: core c -> batch b=c//4, head group g=c%4 (4 heads, d_model
slice [256g, 256g+256)). Each core computes q/k/v projections for its
heads (full X input, sliced weights), causal attention, and a partial
output projection y_partial = sdpa_g @ Wo[:, slice].T. Host sums the 4
partials per batch and adds bo.

v2b: all matmul operands are bf16 (f32r streams 4-byte data at ~1.5
cyc/elem on the PE; bf16 gets 1.0 + fast weight load). Attention runs
in head-pair passes: both heads' scores land in one 2-bank PSUM tile so
a single paired exp evicts them (half the scalar-engine instruction
count). Diagonal key-blocks are column-sliced (masked query columns are
never computed); only the 128x128 triangle blocks need a 0/1 mask.
Softmax denominators come free via a ones-row appended to V; they are
extracted with one copy per head-pair, DMA-reshaped to a [128,8]
layout where the (iterative, ~6cyc/elem) reciprocal is nearly free,
then broadcast back via DRAM. Normalization is 1 in-place bf16 mult
per head-pair on the Wo moving operand. Projection/Wo work for
adjacent chunks is interleaved into the attention instruction stream
so the in-order tensor queue never idles on the scalar engine.
"""
import sys
import os
from collections import deque

sys.path.insert(0, "/opt/trn_rl_repo")

import numpy as np
import ml_dtypes

BF16 = ml_dtypes.bfloat16

H = 16
D = 1024
DK = 64
B, S = 2, 2048
P = 128
SC = 512           # sequence chunk (matmul free dim)
NSC = S // SC      # 4
NKC = D // P       # 8 contraction chunks for projections
FL = 256           # local features per core (4 heads x 64)
NFC = FL // P      # 2
HL = 4             # local heads
NJB = S // P       # 16 key blocks

_state = {}

# Results of the last kernel() call (for test harness inspection)
last_results = None


def _build_nc():
    import concourse.bass as bass
    import concourse.mybir as mybir
    import concourse.tile as tile
    from concourse import bacc

    f32 = mybir.dt.float32
    bf16 = mybir.dt.bfloat16
    AF = mybir.ActivationFunctionType
    MULT = mybir.AluOpType.mult
    ADD = mybir.AluOpType.add
    ts = bass.ts

    nc = bacc.Bacc("TRN2", target_bir_lowering=False, debug=False, num_devices=8)

    # DRAM I/O (per-core shapes; data differs per core)
    xqT = nc.dram_tensor("xqT", [D, S], bf16, kind="ExternalInput")
    xkT = nc.dram_tensor("xkT", [D, S], bf16, kind="ExternalInput")
    xvT = nc.dram_tensor("xvT", [D, S], bf16, kind="ExternalInput")
    wqT = nc.dram_tensor("wqT", [D, FL], bf16, kind="ExternalInput")
    wkT = nc.dram_tensor("wkT", [D, FL], bf16, kind="ExternalInput")
    wvT = nc.dram_tensor("wvT", [D, FL], bf16, kind="ExternalInput")
    woT = nc.dram_tensor("woT", [FL, D], bf16, kind="ExternalInput")
    bqs = nc.dram_tensor("bqs", [P, NFC], f32, kind="ExternalInput")
    bks = nc.dram_tensor("bks", [P, NFC], f32, kind="ExternalInput")
    bvb = nc.dram_tensor("bvb", [P, FL], f32, kind="ExternalInput")
    msk = nc.dram_tensor("msk", [P, 2, P], bf16, kind="ExternalInput")
    onec = nc.dram_tensor("onec", [P, NJB * HL], bf16, kind="ExternalInput")
    yT = nc.dram_tensor("yT", [D, S], bf16, kind="ExternalOutput")
    KDEBUG = bool(os.environ.get("KDEBUG"))
    scr_kind = "ExternalOutput" if KDEBUG else "Internal"
    scrA = nc.dram_tensor("scrA", [NSC * HL, SC], f32, kind=scr_kind)
    scrB = nc.dram_tensor("scrB", [NSC * HL, SC], f32, kind=scr_kind)
    if KDEBUG:
        sdpaD = nc.dram_tensor("sdpaD", [P, NFC, S], bf16, kind="ExternalOutput")

    xq_r = xqT.ap().rearrange("(o p) s -> p o s", p=P)
    xk_r = xkT.ap().rearrange("(o p) s -> p o s", p=P)
    xv_r = xvT.ap().rearrange("(o p) s -> p o s", p=P)
    wq_r = wqT.ap().rearrange("(o p) f -> p o f", p=P)
    wk_r = wkT.ap().rearrange("(o p) f -> p o f", p=P)
    wv_r = wvT.ap().rearrange("(o p) f -> p o f", p=P)
    wo_r = woT.ap().rearrange("(o p) d -> p o d", p=P)
    yT_r = yT.ap().rearrange("(o p) s -> p o s", p=P)

    with tile.TileContext(nc) as tc:
        with tc.tile_pool(name="const", bufs=1) as const, \
             tc.tile_pool(name="xpool", bufs=2) as xpool, \
             tc.tile_pool(name="big", bufs=1) as big, \
             tc.tile_pool(name="work", bufs=4) as work, \
             tc.tile_pool(name="bcp", bufs=3) as bcp, \
             tc.tile_pool(name="rcp", bufs=2) as rcp, \
             tc.tile_pool(name="ytile", bufs=4) as ytile, \
             tc.tile_pool(name="pp", bufs=2, space="PSUM") as pp, \
             tc.tile_pool(name="pss", bufs=2, space="PSUM") as pss, \
             tc.tile_pool(name="pso", bufs=1, space="PSUM") as pso, \
             tc.tile_pool(name="pso2", bufs=1, space="PSUM") as pso2:

            # ---- constants ----
            w_q = const.tile([P, NKC, FL], bf16, tag="wq")
            w_k = const.tile([P, NKC, FL], bf16, tag="wk")
            w_v = const.tile([P, NKC, FL], bf16, tag="wv")
            w_o = const.tile([P, NFC, D], bf16, tag="wo")
            nc.sync.dma_start(w_q[:], wq_r)
            nc.sync.dma_start(w_k[:], wk_r)
            nc.sync.dma_start(w_v[:], wv_r)
            nc.sync.dma_start(w_o[:], wo_r)
            b_q = const.tile([P, NFC], f32, tag="bq")
            b_k = const.tile([P, NFC], f32, tag="bk")
            b_v = const.tile([P, FL], f32, tag="bv")
            nc.sync.dma_start(b_q[:], bqs.ap())
            nc.sync.dma_start(b_k[:], bks.ap())
            nc.sync.dma_start(b_v[:], bvb.ap())
            # 128x128 causal triangle (x >= p), replicated for head pairs
            tri = const.tile([P, 2, P], bf16, tag="tri")
            nc.sync.dma_start(tri[:], msk.ap())

            # ---- persistent intermediates (all bf16) ----
            kT = big.tile([P, NFC, S], bf16, tag="kT")
            qT = big.tile([P, NFC, S], bf16, tag="qT")
            vaug = big.tile([P, NJB, HL * (DK + 1)], bf16, tag="vaug")
            sdpaU = big.tile([P, NFC, S], bf16, tag="sdpaU")
            # ones column per head at position 64 within each 65-wide group
            ones_dst = vaug[:].rearrange("p j (h u) -> p j h u", u=DK + 1)[
                :, :, :, DK
            ]
            nc.sync.dma_start(ones_dst, onec.ap().rearrange("p (j h) -> p j h", h=HL))

            pend = {}

            def dma_x(c):
                # bulk input loads ride the GpSimd swdge queue so the sync
                # ring stays free for the latency-critical denominator
                # chain (FIFO ring: a 3us xt transfer ahead of a chain DMA
                # adds ~10us to every normalize)
                tiles = {}
                for name, x_r in (("k", xk_r), ("q", xq_r), ("v", xv_r)):
                    xt = xpool.tile([P, NKC, SC], bf16, tag=f"x{name}",
                                    name=f"x{name}_{c}")
                    nc.gpsimd.dma_start(xt[:], x_r[:, :, ts(c, SC)])
                    tiles[name] = xt
                return tiles

            def proj_tasks(c, xts):
                tasks = []
                for name, w_t, b_t, outT in (("k", w_k, b_k, kT),
                                             ("q", w_q, b_q, qT)):
                    for fc in range(NFC):
                        def t(name=name, w_t=w_t, b_t=b_t, outT=outT, fc=fc,
                              xt=xts[name], c=c):
                            ps = pp.tile([P, SC], f32, tag="p512",
                                         name=f"pp{name}_{c}_{fc}")
                            for k in range(NKC):
                                nc.tensor.matmul(
                                    ps[:], w_t[:, k, ts(fc, P)], xt[:, k, :],
                                    start=(k == 0), stop=(k == NKC - 1),
                                )
                            nc.vector.tensor_scalar_add(
                                outT[:, fc, ts(c, SC)], ps[:], b_t[:, fc:fc + 1]
                            )
                        tasks.append(t)
                for sb in range(SC // P):
                    def t(sb=sb, xt=xts["v"], c=c):
                        j = c * 4 + sb
                        ps = pp.tile([P, SC], f32, tag="p512", name=f"ppv_{j}")
                        for k in range(NKC):
                            nc.tensor.matmul(
                                ps[:, :FL], xt[:, k, ts(sb, P)], w_v[:, k, :],
                                start=(k == 0), stop=(k == NKC - 1),
                            )
                        dst = vaug[:, j].rearrange("p (h u) -> p h u", u=DK + 1)[
                            :, :, :DK
                        ]
                        src = ps[:, :FL].rearrange("p (h u) -> p h u", u=DK)
                        bsrc = b_v[:].rearrange("p (h u) -> p h u", u=DK)
                        nc.vector.tensor_tensor(dst, src, bsrc, ADD)
                    tasks.append(t)
                return tasks

            def normwo_tasks(c):
                tasks = []
                for fc in range(NFC):
                    bc = pend.pop((c, fc))
                    def t(fc=fc, c=c, bc=bc):
                        sl = sdpaU[:, fc, ts(c, SC)]
                        nc.vector.tensor_tensor(sl, sl, bc[:], MULT)
                    tasks.append(t)
                for mo in range(D // P):
                    def t(mo=mo, c=c):
                        ps_y = pp.tile([P, SC], f32, tag="p512",
                                       name=f"py_{c}_{mo}")
                        for fc in range(NFC):
                            nc.tensor.matmul(
                                ps_y[:], w_o[:, fc, ts(mo, P)],
                                sdpaU[:, fc, ts(c, SC)],
                                start=(fc == 0), stop=(fc == NFC - 1),
                                skip_group_check=True,
                            )
                        ys = ytile.tile([P, SC], bf16, tag="y",
                                        name=f"ys_{c}_{mo}")
                        nc.vector.tensor_copy(ys[:], ps_y[:])
                        # y DMAs ride the idle GpSimd queue: the sync queue
                        # must not head-of-line block them behind the
                        # denominator DMA chain (stalls cascade into DVE)
                        nc.gpsimd.dma_start(yT_r[:, mo, ts(c, SC)], ys[:])
                    tasks.append(t)
                return tasks

            def attention(c, fillers):
                n_j = 4 * c + 4
                j_order = list(range(4 * c, n_j)) + list(range(4 * c))
                for hp in range(2):
                    filler = fillers[hp]
                    # two 1-bank tiles so each head's bank is released to
                    # the next head-pair as soon as its own evicts finish
                    po = [pso.tile([P, SC], f32, tag="po0",
                                   name=f"po_{c}_{hp}_0"),
                          pso2.tile([P, SC], f32, tag="po1",
                                    name=f"po_{c}_{hp}_1")]

                    def emit_attnv(jx, j, et, off, hp=hp, po=po):
                        for hh in range(2):
                            h = 2 * hp + hh
                            nc.tensor.matmul(
                                po[hh][0:DK + 1, off:],
                                vaug[:, j, (DK + 1) * h:(DK + 1) * (h + 1)],
                                et[:, hh, off:],
                                start=(jx == 0), stop=(jx == n_j - 1),
                                skip_group_check=True,
                            )

                    prev = None
                    for jx, j in enumerate(j_order):
                        jd = j - 4 * c if j >= 4 * c else None
                        off = P * jd if jd is not None else 0
                        ps_s = pss.tile([P, 2, SC], f32, tag="ps")
                        for hh in range(2):
                            nc.tensor.matmul(
                                ps_s[:, hh, off:],
                                kT[DK * hh:DK * (hh + 1), hp, ts(j, P)],
                                qT[DK * hh:DK * (hh + 1), hp,
                                   c * SC + off:(c + 1) * SC],
                                start=True, stop=True, skip_group_check=True,
                            )
                        et = work.tile([P, 2, SC], bf16, tag="et")
                        nc.scalar.activation(
                            et[:, :, off:], ps_s[:, :, off:], AF.Exp
                        )
                        if jd is not None:
                            nc.vector.tensor_tensor(
                                et[:, :, off:off + P],
                                et[:, :, off:off + P],
                                tri[:], MULT,
                            )
                        # one-j skew: attnV(j-1) fills the tensor queue
                        # while exp(j) runs on the scalar engine
                        if prev is not None:
                            emit_attnv(*prev)
                        if filler:
                            filler.popleft()()
                        prev = (jx, j, et, off)
                    emit_attnv(*prev)
                    # evict unnormalized sdpa; denominators -> [128,8] via
                    # DRAM reshape, reciprocal there, broadcast back.
                    # Per-head order (z, dn) releases each bank asap.
                    r0 = 2 * (2 * c + hp)
                    for hh in range(2):
                        nc.vector.tensor_copy(
                            sdpaU[DK * hh:DK * (hh + 1), hp, ts(c, SC)],
                            po[hh][0:DK, :],
                        )
                        dn = rcp.tile([1, SC], f32, tag=f"dn{hh}",
                                      name=f"dn_{c}_{hp}_{hh}")
                        nc.vector.tensor_copy(dn[:], po[hh][DK:DK + 1, :])
                        nc.sync.dma_start(
                            scrA.ap()[r0 + hh:r0 + hh + 1, :], dn[:]
                        )
                    shuf = "h (qh ql) -> ql (h qh)"
                    dn2 = rcp.tile([P, 2 * SC // P], f32, tag="dn2",
                                   name=f"dn2_{c}_{hp}")
                    nc.sync.dma_start(
                        dn2[:], scrA.ap()[r0:r0 + 2, :].rearrange(shuf, ql=P)
                    )
                    rc2 = rcp.tile([P, 2 * SC // P], f32, tag="rc2",
                                   name=f"rc2_{c}_{hp}")
                    nc.vector.reciprocal(rc2[:], dn2[:])
                    nc.sync.dma_start(
                        scrB.ap()[r0:r0 + 2, :].rearrange(shuf, ql=P), rc2[:]
                    )
                    bc = bcp.tile([P, SC], f32, tag="bc", name=f"bc_{c}_{hp}")
                    for hh in range(2):
                        nc.sync.dma_start(
                            bc[DK * hh:DK * (hh + 1), :],
                            scrB.ap()[r0 + hh:r0 + hh + 1, :]
                            .to_broadcast((DK, SC)),
                        )
                    pend[(c, hp)] = bc
                    # drain leftover interleave tasks for this pass (tasks
                    # must never be dropped — that loses output blocks)
                    while filler:
                        filler.popleft()()

            # ---- main schedule ----
            xts = dma_x(0)
            for t in proj_tasks(0, xts):
                t()
            for c in range(NSC):
                if c + 1 < NSC:
                    xts = dma_x(c + 1)
                    p_tasks = proj_tasks(c + 1, xts)
                else:
                    p_tasks = []
                nw = normwo_tasks(c - 1) if c > 0 else []
                # norm(c-1,hp0)'s bc chain finished long ago -> hp0 pass;
                # norm(c-1,hp1)'s chain launched at the end of chunk c-1,
                # needs ~10us -> delay it (and the Wo tasks that depend on
                # both norms) to the hp1 pass so it never head-of-line
                # blocks the DVE queue.
                f0 = deque(p_tasks[:4] + nw[0:1] + p_tasks[4:])
                f1 = deque(nw[1:2] + nw[2:])
                attention(c, [f0, f1])
            for t in normwo_tasks(NSC - 1):
                t()
            if KDEBUG:
                nc.sync.dma_start(sdpaD.ap(), sdpaU[:])

    nc.compile()
    return nc


def _get_nc():
    if "nc" not in _state:
        _state["nc"] = _build_nc()
    return _state["nc"]


def kernel(Q, K, V, mask, Wq, bq, Wk, bk, Wv, bv, Wo, bo):
    global last_results
    from concourse.bass_utils import run_bass_kernel_spmd

    Q = np.asarray(Q, np.float32)
    K = np.asarray(K, np.float32)
    V = np.asarray(V, np.float32)
    Wq = np.asarray(Wq, np.float32)
    bq = np.asarray(bq, np.float32)
    Wk = np.asarray(Wk, np.float32)
    bk = np.asarray(bk, np.float32)
    Wv = np.asarray(Wv, np.float32)
    bv = np.asarray(bv, np.float32)
    Wo = np.asarray(Wo, np.float32)
    bo = np.asarray(bo, np.float32)

    nc = _get_nc()

    # causal triangle mask (x >= p), replicated for the 2-head exp pair
    p = np.arange(P)[:, None, None]
    x = np.arange(P)[None, None, :]
    msk_np = np.broadcast_to(x >= p, (P, 2, P)).astype(BF16)

    xT = {}
    for b in range(B):
        xT[("q", b)] = Q[b].T.astype(BF16)
        xT[("k", b)] = K[b].T.astype(BF16)
        xT[("v", b)] = V[b].T.astype(BF16)

    in_maps = []
    for core in range(8):
        b = core // 4
        g = core % 4
        fs, fe = FL * g, FL * (g + 1)
        # fold the 1/sqrt(dk)=0.125 score scale into the q side (exact)
        wq_s = (Wq[fs:fe, :] * 0.125).T.astype(BF16)
        bq_s = bq[fs:fe] * 0.125
        in_maps.append({
            "xqT": xT[("q", b)],
            "xkT": xT[("k", b)],
            "xvT": xT[("v", b)],
            "wqT": wq_s,
            "wkT": Wk[fs:fe, :].T.astype(BF16),
            "wvT": Wv[fs:fe, :].T.astype(BF16),
            "woT": Wo[:, fs:fe].T.astype(BF16),
            "bqs": np.ascontiguousarray(bq_s.reshape(NFC, P).T),
            "bks": np.ascontiguousarray(bk[fs:fe].reshape(NFC, P).T),
            "bvb": np.ascontiguousarray(
                np.broadcast_to(bv[fs:fe][None, :], (P, FL))
            ).astype(np.float32),
            "msk": msk_np,
            "onec": np.ones((P, NJB * HL), BF16),
        })

    res = run_bass_kernel_spmd(nc, in_maps, core_ids=list(range(8)))
    last_results = res

    out = np.empty((B, S, D), np.float32)
    for b in range(B):
        acc = res.results[4 * b]["yT"].astype(np.float32)
        for g in range(1, 4):
            acc = acc + res.results[4 * b + g]["yT"].astype(np.float32)
        out[b] = acc.T + bo[None, :]
    return out
